# revision 23
# baseline (speedup 1.0000x reference)
"""AdaptiveGraphConv on 8 TRN2 NeuronCores (Bass/Tile).

Strategy: destination-sharded edge-parallel.  Edges are sorted by dst and
sharded by dst-node range (NLOC nodes/core), so each core owns the full
aggregation for its dst range and NO all-reduce of the [N, D] aggregate is
needed.  Node features h (and A = h@We1_top + be1) are computed node-parallel
and exchanged with an AllGather of a bf16 [h|A] table that is CHUNKED along
the node axis and overlapped with the phase-1 compute (the table row order is
chunk-major, which the host folds into the gather indices).  B = h@We1_bot is
only ever indexed by dst (always core-local, kept resident in SBUF).

Per-edge work uses dma_gather (SWDGE, 4 queues) for [h|A][src].  The edge-MLP
score runs on the Vector engine in [slot, feat] layout with large batches:
z = A[src] + Bf (Bf is a per-tile B table materialized once per chunk:
broadcast of the resident B for aligned tiles, one-hot-T matmuls for
overflow tiles), then fused relu*We2 and a free-axis reduce.  The weighted
segment scatter-add is ONE matmul per 128-edge tile:
aggr += h_tile^T @ (pattern * w); patterns (identity for aligned tiles,
one-hot for overflow) are streamed from HBM and multiplied by w in one
batched Vector op per (group, half) via a stride-0 broadcast of w.

dma_gather indices are signed int16, so src gathers run against two table
views (rows [0,32768) and [32768,NPAD)); every tile is lo/hi by src id.
Chunk tile order: [g0 alo, g0 ovlo, g1 alo, g1 ovlo][g0 ahi, g0 ovhi, ...].

The gate/blend epilogue is interleaved per dst-group into the main edge loop
(Scalar stays on the Sigmoid activation table throughout); the final
LayerNorm apply is batched after the loop so Sqrt loads its table once.
"""

import sys
import types

sys.path.insert(0, "/opt/trn_rl_repo")

import math

import numpy as np
import ml_dtypes

import concourse.bass as bass
import concourse.bacc as bacc
import concourse.tile as tile
from concourse import mybir
from concourse.bass_utils import run_bass_kernel_spmd

BF16 = ml_dtypes.bfloat16
F32 = mybir.dt.float32
BF = mybir.dt.bfloat16
I16 = mybir.dt.int16

N_CORES = 8
D = 128
P = 128
SPLIT = 32768      # int16 index limit for dma_gather
CHUNK_G = 2        # groups per gather/compute chunk
SB = 8             # score-batch: tiles per PSUM z build
GS = 16            # tiles per dma_gather call
WOV = 1.25         # overflow-tile weight in region packing


def _install_ntff_hook():
    if "antenv.axon_hooks" in sys.modules:
        return
    try:
        from trn_agent_boot.trn_boot import _ntff_profile_via_ctypes

        hook = _ntff_profile_via_ctypes("/opt/axon/libaxon_pjrt.so")
    except Exception:
        hook = None
    mod = types.ModuleType("antenv.axon_hooks")
    mod.get_axon_ntff_profile_hook = lambda: hook
    mod.set_axon_ntff_profile_hook = lambda h: None
    sys.modules["antenv.axon_hooks"] = mod


# ----------------------------------------------------------------------------
# device program
# ----------------------------------------------------------------------------

def _build_program(NG, NLOC, NPAD, T_alo, T_ahi, T_ovlo, T_ovhi, cc_bounds, npos):
    T_lo = [a + o for a, o in zip(T_alo, T_ovlo)]
    T_hi = [a + o for a, o in zip(T_ahi, T_ovhi)]
    T_ov = [a + b for a, b in zip(T_ovlo, T_ovhi)]
    T_all = [a + b for a, b in zip(T_lo, T_hi)]
    NT = sum(T_all)
    NTlo = sum(T_lo)
    NThi = sum(T_hi)
    hi_base = min(SPLIT, NPAD)

    nc = bacc.Bacc(
        "TRN2", target_bir_lowering=False, debug=False, num_devices=N_CORES,
        num_swdge_queues=4,
    )

    def din(name, shape, dt):
        return nc.dram_tensor(name, list(shape), dt, kind="ExternalInput").ap()

    xT = din("xT", [P, NLOC], F32)
    W1 = din("W1", [P, D], F32)
    b1row = din("b1row", [1, D], F32)
    ones1f = din("ones1f", [1, D], F32)
    ones1b = din("ones1b", [1, D], BF)
    g1b = din("g1b", [P, D], F32)
    bt1b = din("bt1b", [P, D], F32)
    We1T = din("We1T", [P, D], BF)
    We1B = din("We1B", [P, D], BF)
    be1row = din("be1row", [1, D], BF)
    be2c = din("be2c", [P, 1], F32)
    WgT = din("WgT", [P, D], BF)
    WgB = din("WgB", [P, D], BF)
    bgc = din("bgc", [P, 1], F32)
    g2b = din("g2b", [P, D], F32)
    bt2b = din("bt2b", [P, D], F32)
    idn = din("idn", [P, P], BF)
    ixlo = din("ixlo", [P, max(NTlo, 1) * 8], I16)
    ixhi = din("ixhi", [P, max(NThi, 1) * 8], I16)
    ohp = din("ohp", [P, max(NT, 1) * P], BF)
    ohTp = din("ohTp", [P, max(sum(T_ov), 1) * P], BF)
    maskb = din("maskb", [P, max(NT, 1)], F32)

    out = nc.dram_tensor("out", [NLOC, D], F32, kind="ExternalOutput").ap()

    chunks = []
    g0 = 0
    while g0 < NG:
        chunks.append((g0, min(g0 + CHUNK_G, NG)))
        g0 = min(g0 + CHUNK_G, NG)
    TloC = max(sum(T_lo[a:b]) for a, b in chunks)
    ThiC = max(sum(T_hi[a:b]) for a, b in chunks)
    TtotC = max(sum(T_all[a:b]) for a, b in chunks)
    TovCk = max(sum(T_ov[a:b]) for a, b in chunks)
    OVC = max(max(T_ovlo), max(T_ovhi))

    with tile.TileContext(nc, trace_sim=False) as tc:
        with (
            tc.tile_pool(name="singles", bufs=1) as sg,
            tc.tile_pool(name="dram", bufs=1, space="DRAM") as dram,
        ):
            def load(ap_in, shape, dt, name):
                t = sg.tile(list(shape), dt, name=name)
                nc.sync.dma_start(out=t[:], in_=ap_in[:])
                return t

            W1_sb = load(W1, [P, D], F32, "W1_sb")
            b1_sb = load(b1row, [1, D], F32, "b1_sb")
            o1f_sb = load(ones1f, [1, D], F32, "o1f_sb")
            o1b_sb = load(ones1b, [1, D], BF, "o1b_sb")
            g1_sb = load(g1b, [P, D], F32, "g1_sb")
            bt1_sb = load(bt1b, [P, D], F32, "bt1_sb")
            We1T_sb = load(We1T, [P, D], BF, "We1T_sb")
            We1B_sb = load(We1B, [P, D], BF, "We1B_sb")
            be1_sb = load(be1row, [1, D], BF, "be1_sb")
            be2_sb = load(be2c, [P, 1], F32, "be2_sb")
            WgT_sb = load(WgT, [P, D], BF, "WgT_sb")
            WgB_sb = load(WgB, [P, D], BF, "WgB_sb")
            bg_sb = load(bgc, [P, 1], F32, "bg_sb")
            g2_sb = load(g2b, [P, D], F32, "g2_sb")
            bt2_sb = load(bt2b, [P, D], F32, "bt2_sb")
            idn_sb = load(idn, [P, P], BF, "idn_sb")
            ixlo_sb = load(ixlo, [P, max(NTlo, 1) * 8], I16, "ixlo_sb")
            ixhi_sb = load(ixhi, [P, max(NThi, 1) * 8], I16, "ixhi_sb")
            mkb_sb = load(maskb, [P, max(NT, 1)], F32, "mkb_sb")

            eps_sb = sg.tile([P, 1], F32, name="eps_sb")
            nc.vector.memset(eps_sb[:], 1e-5)

            hT_sb = sg.tile([P, NLOC], BF, name="hT_sb")
            B_sb = sg.tile([P, NLOC], BF, name="B_sb")
            hnT_sb = sg.tile([P, NLOC], BF, name="hnT_sb")
            mvall = sg.tile([P, 2 * NG], F32, name="mvall")

            HA_shard = dram.tile([NLOC, 2 * D], BF, name="HA_shard")
            n_cc = len(cc_bounds) - 1
            cc_rows = [
                (cc_bounds[k] * P * N_CORES, cc_bounds[k + 1] * P * N_CORES)
                for k in range(n_cc)
            ]
            HA_cc = [
                dram.tile(
                    [cc_rows[k][1] - cc_rows[k][0], 2 * D],
                    BF, name=f"HA_cc{k}", addr_space="Shared",
                )
                for k in range(n_cc)
            ]
            HA_lo_d = dram.tile([hi_base, 2 * D], BF, name="HA_lo_d")
            n_hi_rows = max(NPAD - hi_base, 1)
            HA_hi_d = dram.tile([n_hi_rows, 2 * D], BF, name="HA_hi_d")

            ha_v = HA_shard.rearrange("(g p) c -> p g c", p=P)

            cp_engs = [nc.sync, nc.scalar]

            def issue_collective(kc):
                """AllGather collective chunk kc, then copy into the
                zero-offset lo / hi gather tables (chunk 0 writes the lo
                table directly when it covers exactly [0, hi_base))."""
                r0, r1 = cc_bounds[kc] * P, cc_bounds[kc + 1] * P
                f0, f1 = r0 * N_CORES, r1 * N_CORES
                nc.gpsimd.collective_compute(
                    "AllGather",
                    mybir.AluOpType.bypass,
                    replica_groups=[list(range(N_CORES))],
                    ins=[HA_shard[r0:r1, :].opt()],
                    outs=[HA_cc[kc][:].opt()],
                )
                l1 = min(f1, hi_base)
                if f0 < l1:
                    cp_engs[(2 * kc) % 2].dma_start(
                        out=HA_lo_d[f0:l1, :],
                        in_=HA_cc[kc][0 : l1 - f0, :],
                    )
                h0 = max(f0, hi_base)
                if h0 < f1:
                    cp_engs[(2 * kc + 1) % 2].dma_start(
                        out=HA_hi_d[h0 - hi_base : f1 - hi_base, :],
                        in_=HA_cc[kc][h0 - f0 : f1 - f0, :],
                    )

            # ================= phase 1: node transform ==================
            with (
                tc.tile_pool(name="xtp", bufs=1) as xtp,
                tc.tile_pool(name="ps1", bufs=2, space="PSUM") as ps1,
                tc.tile_pool(name="w1p", bufs=3) as w1p,
            ):
                xT_sb = xtp.tile([P, NLOC], F32, name="xT_sb")
                nc.sync.dma_start(out=xT_sb[:], in_=xT[:])
                h_sb = xtp.tile([P, NLOC], BF, name="h_sb")
                A_sb = xtp.tile([P, NLOC], BF, name="A_sb")
                for g in range(NG):
                    gsl = slice(g * P, (g + 1) * P)
                    hp = ps1.tile([P, D], F32, tag="hpre", name=f"hp{g}")
                    nc.tensor.matmul(
                        out=hp[:], lhsT=xT_sb[:, gsl], rhs=W1_sb[:],
                        start=True, stop=False,
                    )
                    nc.tensor.matmul(
                        out=hp[:], lhsT=o1f_sb[:], rhs=b1_sb[:],
                        start=False, stop=True,
                    )
                    st = w1p.tile([P, 6], F32, tag="st", name=f"st{g}")
                    nc.vector.bn_stats(out=st[:], in_=hp[:])
                    mv = w1p.tile([P, 2], F32, tag="mv", name=f"mv{g}")
                    nc.vector.bn_aggr(out=mv[:], in_=st[:])
                    sd = w1p.tile([P, 1], F32, tag="sd", name=f"sd{g}")
                    nc.scalar.activation(
                        out=sd[:], in_=mv[:, 1:2],
                        func=mybir.ActivationFunctionType.Sqrt,
                        bias=eps_sb[:],
                    )
                    rstd = w1p.tile([P, 1], F32, tag="rstd", name=f"rs{g}")
                    nc.vector.reciprocal(out=rstd[:], in_=sd[:])
                    t1 = w1p.tile([P, D], F32, tag="t1", name=f"t1{g}")
                    nc.vector.tensor_scalar(
                        out=t1[:], in0=hp[:], scalar1=mv[:, 0:1],
                        scalar2=rstd[:], op0=mybir.AluOpType.subtract,
                        op1=mybir.AluOpType.mult,
                    )
                    u1 = w1p.tile([P, D], F32, tag="u1", name=f"u1{g}")
                    nc.vector.tensor_mul(out=u1[:], in0=t1[:], in1=g1_sb[:])
                    v1 = w1p.tile([P, D], F32, tag="v1", name=f"v1{g}")
                    nc.vector.tensor_add(out=v1[:], in0=u1[:], in1=bt1_sb[:])
                    nc.vector.tensor_scalar_max(
                        out=h_sb[:, gsl], in0=v1[:], scalar1=0.0
                    )
                    htp = ps1.tile([P, D], F32, tag="hT", name=f"htp{g}")
                    nc.tensor.matmul(
                        out=htp[:], lhsT=h_sb[:, gsl], rhs=idn_sb[:],
                        start=True, stop=True,
                    )
                    nc.any.tensor_copy(out=hT_sb[:, gsl], in_=htp[:])
                    ap_ = ps1.tile([P, D], F32, tag="A", name=f"apz{g}")
                    nc.tensor.matmul(
                        out=ap_[:], lhsT=hT_sb[:, gsl], rhs=We1T_sb[:],
                        start=True, stop=False,
                    )
                    nc.tensor.matmul(
                        out=ap_[:], lhsT=o1b_sb[:], rhs=be1_sb[:],
                        start=False, stop=True,
                    )
                    nc.any.tensor_copy(out=A_sb[:, gsl], in_=ap_[:])
                    bp = ps1.tile([P, D], F32, tag="B", name=f"bp{g}")
                    nc.tensor.matmul(
                        out=bp[:], lhsT=hT_sb[:, gsl], rhs=We1B_sb[:],
                        start=True, stop=True,
                    )
                    nc.any.tensor_copy(out=B_sb[:, gsl], in_=bp[:])

                    if (g + 1) in cc_bounds:
                        kc = cc_bounds.index(g + 1) - 1
                        cg = slice(cc_bounds[kc], cc_bounds[kc + 1])
                        nc.sync.dma_start(
                            out=ha_v[:, cg, 0:D],
                            in_=h_sb.rearrange(
                                "p (g j) -> p g j", g=NG
                            )[:, cg, :],
                        )
                        nc.sync.dma_start(
                            out=ha_v[:, cg, D : 2 * D],
                            in_=A_sb.rearrange(
                                "p (g j) -> p g j", g=NG
                            )[:, cg, :],
                        )
                        issue_collective(kc)

            # ============ phase 2+3: edges + gate/blend =================
            with (
                tc.tile_pool(name="pag", bufs=2, space="PSUM") as pag,
                tc.tile_pool(name="pbv", bufs=2, space="PSUM") as pbv,
                tc.tile_pool(name="pg3", bufs=1, space="PSUM") as pg3,
                tc.tile_pool(name="gio", bufs=3) as gio,
                tc.tile_pool(name="ohp_p", bufs=2) as ohpp,
                tc.tile_pool(name="wrk", bufs=2) as wrk,
                tc.tile_pool(name="mrk", bufs=3) as mrk,
                tc.tile_pool(name="mtp", bufs=2) as mtp,
                tc.tile_pool(name="osb", bufs=2) as osb,
            ):
                lo_off = 0
                hi_off = 0
                b_off = 0
                t_off = 0
                qctr = [0]
                var_v = mvall.rearrange("p (g two) -> p g two", two=2)
                sdall = sg.tile([P, NG], F32, name="sdall")
                rstdall = sg.tile([P, NG], F32, name="rstdall")

                def ln_apply(g0, g1, half):
                    """Final LayerNorm apply for groups [g0, g1)."""
                    nc.scalar.activation(
                        out=sdall[:, g0:g1], in_=var_v[:, g0:g1, 1],
                        func=mybir.ActivationFunctionType.Sqrt,
                        bias=eps_sb[:],
                    )
                    nc.vector.reciprocal(
                        out=rstdall[:, g0:g1], in_=sdall[:, g0:g1]
                    )
                    for g in range(g0, g1):
                        gsl = slice(g * P, (g + 1) * P)
                        t1o = osb.tile([P, D], F32, tag="t1o",
                                       name=f"t1o{g}")
                        nc.vector.tensor_scalar(
                            out=t1o[:], in0=hnT_sb[:, gsl],
                            scalar1=mvall[:, 2 * g : 2 * g + 1],
                            scalar2=rstdall[:, g : g + 1],
                            op0=mybir.AluOpType.subtract,
                            op1=mybir.AluOpType.mult,
                        )
                        u1o = osb.tile([P, D], F32, tag="u1o",
                                       name=f"u1o{g}")
                        nc.vector.tensor_mul(out=u1o[:], in0=t1o[:],
                                             in1=g2_sb[:])
                        o1o = osb.tile([P, D], F32, tag="o1o",
                                       name=f"o1o{g}")
                        nc.vector.tensor_add(out=o1o[:], in0=u1o[:],
                                             in1=bt2_sb[:])
                        nc.sync.dma_start(out=out[gsl, :], in_=o1o[:])

                def next_q():
                    q = qctr[0] % 4
                    qctr[0] += 1
                    return q

                def gather_sliced(buf, table, ix_sb_t, off, ntiles, elem):
                    for s0 in range(0, ntiles, GS):
                        s1 = min(s0 + GS, ntiles)
                        nc.gpsimd.dma_gather(
                            out_ap=buf[:, s0:s1, :],
                            in_ap=table,
                            idxs_ap=ix_sb_t[:, (off + s0) * 8 : (off + s1) * 8],
                            num_idxs=(s1 - s0) * P,
                            num_idxs_reg=(s1 - s0) * P,
                            elem_size=elem,
                            single_packet=False,
                            queue_num=next_q(),
                        )

                for (ga, gb) in chunks:
                    nlo = sum(T_lo[ga:gb])
                    nhi = sum(T_hi[ga:gb])
                    nov = sum(T_ov[ga:gb])
                    ntot = nlo + nhi
                    halo = gio.tile(
                        [P, max(TloC, 1), 2 * D], BF, tag="halo",
                        name=f"halo{ga}",
                    )
                    hahi = gio.tile(
                        [P, max(ThiC, 1), 2 * D], BF, tag="hahi",
                        name=f"hahi{ga}",
                    )
                    ohc = ohpp.tile(
                        [P, max(TtotC, 1), P], BF, tag="ohc", name=f"ohc{ga}"
                    )
                    ohTc = ohpp.tile(
                        [P, max(TovCk, 1), P], BF, tag="ohTc",
                        name=f"ohTc{ga}",
                    )
                    if nlo:
                        gather_sliced(halo, HA_lo_d[:, :], ixlo_sb,
                                      lo_off, nlo, 2 * D)
                    if nhi:
                        gather_sliced(hahi, HA_hi_d[:, :], ixhi_sb,
                                      hi_off, nhi, 2 * D)
                    nc.sync.dma_start(
                        out=ohc[:, 0:ntot, :],
                        in_=ohp[:, t_off * P : (t_off + ntot) * P].rearrange(
                            "p (t j) -> p t j", j=P
                        ),
                    )
                    if nov:
                        nc.sync.dma_start(
                            out=ohTc[:, 0:nov, :],
                            in_=ohTp[:, b_off * P : (b_off + nov) * P].rearrange(
                                "p (t j) -> p t j", j=P
                            ),
                        )

                    # per-group offsets within the chunk
                    off_lo, off_hi, off_ov = {}, {}, {}
                    pl = ph = pv = 0
                    for g in range(ga, gb):
                        off_lo[g] = pl
                        off_hi[g] = ph
                        off_ov[g] = pv
                        pl += T_lo[g]
                        ph += T_hi[g]
                        pv += T_ov[g]

                    # ---- per-position info: (group, is_ov, ov chunk idx)
                    def posinfo(half_is_hi):
                        info = []
                        for g in range(ga, gb):
                            al_n = T_ahi[g] if half_is_hi else T_alo[g]
                            ov_n = T_ovhi[g] if half_is_hi else T_ovlo[g]
                            ov0 = off_ov[g] + (T_ovlo[g] if half_is_hi else 0)
                            info += [(g, False, 0)] * al_n
                            info += [(g, True, ov0 + j) for j in range(ov_n)]
                        return info

                    # ---- scores: z = pattern^T@B + A[src] built on TensorE
                    # in PSUM; relu on Scalar; signed-split reduce on Vector
                    s_sb = wrk.tile([P, max(TtotC, 1)], F32, tag="s",
                                    name=f"s{ga}")
                    for (buf, hi_half, cnt, sbase) in (
                        (halo, False, nlo, 0),
                        (hahi, True, nhi, nlo),
                    ):
                        info = posinfo(hi_half)
                        for c0 in range(0, cnt, SB):
                            nb = min(SB, cnt - c0)
                            zp = pbv.tile(
                                [P, SB, D], F32, tag="zp",
                                name=f"zp{ga}_{sbase}_{c0}",
                            )
                            for j in range(nb):
                                g_, is_ov, ovi = info[c0 + j]
                                gsl_ = slice(g_ * P, (g_ + 1) * P)
                                pat = (ohTc[:, ovi, :] if is_ov
                                       else ohc[:, sbase + c0 + j, :])
                                nc.tensor.matmul(
                                    out=zp[:, j, :], lhsT=pat,
                                    rhs=B_sb[:, gsl_],
                                    start=True, stop=False,
                                )
                                nc.tensor.matmul(
                                    out=zp[:, j, :], lhsT=idn_sb[:],
                                    rhs=buf[:, c0 + j, D : 2 * D],
                                    start=False, stop=True,
                                )
                            z = mrk.tile(
                                [P, SB, D], BF, tag="z",
                                name=f"z{ga}_{sbase}_{c0}",
                            )
                            nc.scalar.activation(
                                out=z[:, 0:nb, :], in_=zp[:, 0:nb, :],
                                func=mybir.ActivationFunctionType.Relu,
                            )
                            if 0 < npos < D:
                                spn = mrk.tile(
                                    [P, 2, SB], F32, tag="spn",
                                    name=f"spn{ga}_{sbase}_{c0}",
                                )
                                nc.vector.tensor_reduce(
                                    out=spn[:, 0, 0:nb],
                                    in_=z[:, 0:nb, 0:npos],
                                    axis=mybir.AxisListType.X,
                                    op=mybir.AluOpType.add,
                                )
                                nc.vector.tensor_reduce(
                                    out=spn[:, 1, 0:nb],
                                    in_=z[:, 0:nb, npos:D],
                                    axis=mybir.AxisListType.X,
                                    op=mybir.AluOpType.add,
                                )
                                nc.vector.tensor_tensor(
                                    out=s_sb[:, sbase + c0 : sbase + c0 + nb],
                                    in0=spn[:, 0, 0:nb], in1=spn[:, 1, 0:nb],
                                    op=mybir.AluOpType.subtract,
                                )
                            else:
                                nc.vector.tensor_reduce(
                                    out=s_sb[:, sbase + c0 : sbase + c0 + nb],
                                    in_=z[:, 0:nb, :],
                                    axis=mybir.AxisListType.X,
                                    op=mybir.AluOpType.add,
                                )
                                if npos == 0:
                                    nc.vector.tensor_scalar_mul(
                                        out=s_sb[
                                            :, sbase + c0 : sbase + c0 + nb
                                        ],
                                        in0=s_sb[
                                            :, sbase + c0 : sbase + c0 + nb
                                        ],
                                        scalar1=-1.0,
                                    )

                    # mask padding slots, sigmoid -> edge weights
                    nc.vector.tensor_tensor(
                        out=s_sb[:, 0:ntot], in0=s_sb[:, 0:ntot],
                        in1=mkb_sb[:, t_off : t_off + ntot],
                        op=mybir.AluOpType.add,
                    )
                    w_sb = wrk.tile([P, max(TtotC, 1)], BF, tag="w",
                                    name=f"w{ga}")
                    nc.scalar.activation(
                        out=w_sb[:, 0:ntot], in_=s_sb[:, 0:ntot],
                        func=mybir.ActivationFunctionType.Sigmoid,
                        bias=be2_sb[:],
                    )

                    # ---- m = pattern * w, one batched op per group-half --
                    mtl = mtp.tile(
                        [P, max(TtotC, 1), P], BF, tag="mtl", name=f"mt{ga}"
                    )
                    for g in range(ga, gb):
                        for (s0, cnt) in (
                            (off_lo[g], T_lo[g]),
                            (nlo + off_hi[g], T_hi[g]),
                        ):
                            if not cnt:
                                continue
                            w_in = w_sb[:, s0 : s0 + cnt].rearrange(
                                "p (t o) -> p t o", o=1
                            ).broadcast_to([P, cnt, P])
                            nc.vector.tensor_tensor(
                                out=mtl[:, s0 : s0 + cnt, :],
                                in0=ohc[:, s0 : s0 + cnt, :],
                                in1=w_in, op=mybir.AluOpType.mult,
                            )

                    # ---- scatter + gate + blend per group ----
                    for g in range(ga, gb):
                        Tg = T_all[g]
                        gsl = slice(g * P, (g + 1) * P)
                        aggr = pag.tile([P, P], F32, tag="aggr", name=f"ag{g}")
                        seq = [
                            (halo, off_lo[g], off_lo[g], T_lo[g]),
                            (hahi, off_hi[g], nlo + off_hi[g], T_hi[g]),
                        ]
                        ti = 0
                        for (buf, bst, ms, cnt) in seq:
                            for j in range(cnt):
                                nc.tensor.matmul(
                                    out=aggr[:], lhsT=buf[:, bst + j, 0:D],
                                    rhs=mtl[:, ms + j, :],
                                    start=(ti == 0),
                                    stop=(ti == Tg - 1),
                                )
                                ti += 1

                        ragg = osb.tile([P, P], BF, tag="ragg", name=f"rg{g}")
                        nc.scalar.activation(
                            out=ragg[:], in_=aggr[:],
                            func=mybir.ActivationFunctionType.Copy,
                        )
                        gp = pg3.tile([P, P], F32, tag="gate", name=f"gp{g}")
                        nc.tensor.matmul(
                            out=gp[:], lhsT=WgT_sb[:], rhs=hT_sb[:, gsl],
                            start=True, stop=False,
                        )
                        nc.tensor.matmul(
                            out=gp[:], lhsT=WgB_sb[:], rhs=ragg[:],
                            start=False, stop=True,
                        )
                        gate = osb.tile([P, P], BF, tag="gate_sb",
                                        name=f"gt{g}")
                        nc.scalar.activation(
                            out=gate[:], in_=gp[:],
                            func=mybir.ActivationFunctionType.Sigmoid,
                            bias=bg_sb[:],
                        )
                        d1 = osb.tile([P, P], BF, tag="d1", name=f"d1{g}")
                        nc.vector.tensor_tensor(
                            out=d1[:], in0=ragg[:], in1=hT_sb[:, gsl],
                            op=mybir.AluOpType.subtract,
                        )
                        d2 = osb.tile([P, P], BF, tag="d2", name=f"d2{g}")
                        nc.vector.tensor_mul(out=d2[:], in0=gate[:], in1=d1[:])
                        hn = osb.tile([P, P], BF, tag="hn", name=f"hn{g}")
                        nc.vector.tensor_add(
                            out=hn[:], in0=hT_sb[:, gsl], in1=d2[:]
                        )

                        # transpose hn -> [node, feat], collect LN stats
                        hnp = pg3.tile([P, P], F32, tag="hnp", name=f"hnp{g}")
                        nc.tensor.matmul(
                            out=hnp[:], lhsT=hn[:], rhs=idn_sb[:],
                            start=True, stop=True,
                        )
                        nc.any.tensor_copy(out=hnT_sb[:, gsl], in_=hnp[:])
                        st3 = osb.tile([P, 6], F32, tag="st3", name=f"st3{g}")
                        nc.vector.bn_stats(out=st3[:], in_=hnp[:])
                        nc.vector.bn_aggr(
                            out=mvall[:, 2 * g : 2 * g + 2], in_=st3[:]
                        )

                    lo_off += nlo
                    hi_off += nhi
                    b_off += nov
                    t_off += ntot
                    if ga < NG // 2 <= gb:
                        ln_apply(0, NG // 2, 0)

                # phase 4 emitted in two halves (see ln_apply below)
                ln_apply(NG // 2, NG, 1)

    nc.compile()
    return nc


# ----------------------------------------------------------------------------
# host-side sharding + launch
# ----------------------------------------------------------------------------

_CACHE = {}


def _wrap16(seq):
    """idx i -> [i%16, i//16], replicated to 128 partitions (8 Q7 cores)."""
    n = len(seq)
    if n == 0:
        return np.zeros((P, 8), np.int16)
    assert n % 16 == 0
    blk = np.asarray(seq, np.int16).reshape(-1, 16).T
    return np.tile(blk, (8, 1))


def _pick_region(degs_per_core):
    """degs_per_core: [n_cores, 128].  Pick aligned tile count a minimizing
    a + WOV * max_k ceil(overflow_k / 128); return (a, ov_tiles)."""
    dmax = int(degs_per_core.max()) if degs_per_core.size else 0
    if dmax == 0:
        return 0, 0
    best = (1 << 30, 0, 0)
    for a in range(dmax + 1):
        ov = np.maximum(degs_per_core - a, 0).sum(axis=1)
        ovt = int(math.ceil(ov.max() / P)) if ov.max() else 0
        tot = a + WOV * ovt
        if tot < best[0]:
            best = (tot, a, ovt)
    return best[1], best[2]


def _group_seqs(rg, cg, T_a, T_o, base):
    """Edges (rows rg, group-local cols cg) of one (group, half).
    Returns (row_seq  [ (T_a+T_o)*128 ], colrel_aligned [T_a*128],
    colrel_ov [T_o*128])."""
    r_h = rg - base
    ordh = np.lexsort((r_h, cg))
    r_h, c_h = r_h[ordh], cg[ordh]
    if len(c_h):
        starts = np.r_[0, np.cumsum(np.bincount(c_h, minlength=P))]
        rank = np.arange(len(c_h)) - starts[c_h]
    else:
        rank = np.zeros(0, np.int64)
    al = rank < T_a
    a_rows = np.zeros((T_a, P), np.int32)
    a_mask = np.zeros((T_a, P), bool)
    a_rows[rank[al], c_h[al]] = r_h[al]
    a_mask[rank[al], c_h[al]] = True
    a_cr = np.where(
        a_mask, np.arange(P, dtype=np.float32)[None, :], np.float32(-1.0)
    )
    r_o = r_h[~al]
    c_o = c_h[~al]
    n_o = len(r_o)
    assert n_o <= T_o * P, (n_o, T_o)
    o_rows = np.zeros(T_o * P, np.int32)
    o_cr = np.full(T_o * P, -1.0, np.float32)
    o_rows[:n_o] = r_o
    o_cr[:n_o] = c_o.astype(np.float32)
    return (
        np.concatenate([a_rows.reshape(-1), o_rows]),
        a_cr.reshape(-1), o_cr,
    )


def kernel(
    x, edge_index, W1, b1, g1, bt1, We1, be1, We2, be2,
    Wn1, bn1, Wn2, bn2, Wg, bg, g2, bt2, _trace=False,
):
    x = np.asarray(x, dtype=np.float32)
    N = x.shape[0]
    NG = (N + N_CORES * P - 1) // (N_CORES * P)
    NLOC = NG * P
    NPAD = NLOC * N_CORES

    # collective chunking: chunk 0 covers exactly the int16-addressable lo
    # table rows [0, 32768) so its AllGather output needs no re-layout copy;
    # the remaining (hi) groups split into two smaller chunks.
    G_LO = SPLIT // (N_CORES * P)
    if 0 < G_LO < NG:
        n_hi_g = NG - G_LO
        h1 = n_hi_g // 2
        cc_bounds = [0, G_LO // 2, G_LO, G_LO + h1, NG]
        cc_bounds = sorted(set(cc_bounds))
    else:
        cc_bounds = [0, NG]

    row = np.asarray(edge_index[0], dtype=np.int64)
    col = np.asarray(edge_index[1], dtype=np.int64)
    order = np.argsort(col, kind="stable")
    row_s = row[order].astype(np.int32)
    col_s = col[order].astype(np.int32)
    bounds = np.searchsorted(col_s, np.arange(N_CORES + 1) * NLOC)

    # node id -> chunk-major gathered-table row: within collective chunk k
    # (local rows [b0*P, b1*P)), core c's rows are contiguous
    node_ids = np.arange(NPAD, dtype=np.int64)
    core_of = node_ids // NLOC
    loc_of = node_ids % NLOC
    table_row = np.zeros(NPAD, np.int64)
    gbase = 0
    for k in range(len(cc_bounds) - 1):
        cb0, cb1 = cc_bounds[k] * P, cc_bounds[k + 1] * P
        ch = cb1 - cb0
        msk = (loc_of >= cb0) & (loc_of < cb1)
        table_row[msk] = gbase + core_of[msk] * ch + (loc_of[msk] - cb0)
        gbase += N_CORES * ch

    row_t = table_row[row_s]          # table rows of edge sources

    deg_lo = np.zeros((N_CORES, NLOC), np.int32)
    deg_hi = np.zeros((N_CORES, NLOC), np.int32)
    for k in range(N_CORES):
        lo, hi = bounds[k], bounds[k + 1]
        cl = col_s[lo:hi] - k * NLOC
        ish = row_t[lo:hi] >= SPLIT
        deg_lo[k] = np.bincount(cl[~ish], minlength=NLOC)
        deg_hi[k] = np.bincount(cl[ish], minlength=NLOC)

    T_alo, T_ahi, T_ovlo, T_ovhi = [], [], [], []
    for g in range(NG):
        csl = slice(g * P, (g + 1) * P)
        a, o = _pick_region(deg_lo[:, csl])
        T_alo.append(a)
        T_ovlo.append(o)
        a, o = _pick_region(deg_hi[:, csl])
        T_ahi.append(a)
        T_ovhi.append(o)
        if T_alo[g] + T_ahi[g] + T_ovlo[g] + T_ovhi[g] == 0:
            T_ovlo[g] = 1
    T_lo = [T_alo[g] + T_ovlo[g] for g in range(NG)]
    T_hi = [T_ahi[g] + T_ovhi[g] for g in range(NG)]
    T_all = [T_lo[g] + T_hi[g] for g in range(NG)]
    NT = sum(T_all)
    NTb = sum(T_ovlo) + sum(T_ovhi)

    # fold |We2| into the edge-MLP hidden columns, positives first, so the
    # score becomes  s = sum(relu(z'')[0:npos]) - sum(relu(z'')[npos:D])
    We2f = np.asarray(We2, np.float32).reshape(-1)
    perm = np.argsort(We2f < 0, kind="stable")
    npos = int((We2f >= 0).sum())
    absw = np.abs(We2f[perm])
    We1n = np.asarray(We1, np.float32)[:, perm] * absw[None, :]
    be1n = np.asarray(be1, np.float32).reshape(-1)[perm] * absw

    key = (N, NG, tuple(T_alo), tuple(T_ahi), tuple(T_ovlo), tuple(T_ovhi))
    if key not in _CACHE:
        _CACHE[key] = _build_program(
            NG, NLOC, NPAD, T_alo, T_ahi, T_ovlo, T_ovhi, cc_bounds, npos
        )
    nc = _CACHE[key]

    bf = lambda a: np.ascontiguousarray(np.asarray(a, np.float32)).astype(BF16)
    f32 = lambda a: np.ascontiguousarray(np.asarray(a, np.float32))
    shared = {
        "W1": f32(W1),
        "b1row": f32(b1).reshape(1, D),
        "ones1f": np.ones((1, D), np.float32),
        "ones1b": np.ones((1, D), BF16),
        "g1b": np.broadcast_to(f32(g1).reshape(1, D), (P, D)).copy(),
        "bt1b": np.broadcast_to(f32(bt1).reshape(1, D), (P, D)).copy(),
        "We1T": bf(We1n[:D]),
        "We1B": bf(We1n[D:]),
        "be1row": bf(be1n).reshape(1, D),
        "be2c": np.broadcast_to(f32(be2).reshape(1, 1), (P, 1)).copy(),
        "WgT": bf(Wg[:D]),
        "WgB": bf(Wg[D:]),
        "bgc": f32(bg).reshape(P, 1),
        "g2b": np.broadcast_to(f32(g2).reshape(1, D), (P, D)).copy(),
        "bt2b": np.broadcast_to(f32(bt2).reshape(1, D), (P, D)).copy(),
        "idn": np.eye(P, dtype=BF16),
    }

    xp = np.zeros((NPAD, D), np.float32)
    xp[:N] = x

    iota_row = np.arange(P, dtype=np.float32)[None, :]
    eye_bf = np.eye(P, dtype=BF16)

    chunks = []
    g0 = 0
    while g0 < NG:
        chunks.append((g0, min(g0 + CHUNK_G, NG)))
        g0 = min(g0 + CHUNK_G, NG)

    in_maps = []
    for k in range(N_CORES):
        lo, hi = bounds[k], bounds[k + 1]
        rk = row_t[lo:hi]
        ck = col_s[lo:hi] - k * NLOC
        gk = ck // P
        # per-group sequences
        g_lo_rows, g_hi_rows = [], []
        g_lo_cr, g_hi_cr = [], []      # aligned+ov colrel per half
        g_ov_cr = []                   # ovlo then ovhi colrel (for ohT)
        for g in range(NG):
            gmask = gk == g
            rg = rk[gmask]
            cg = (ck[gmask] - g * P).astype(np.int64)
            ishg = rg >= SPLIT
            lo_rows, lo_acr, lo_ocr = _group_seqs(
                rg[~ishg], cg[~ishg], T_alo[g], T_ovlo[g], 0
            )
            hi_rows, hi_acr, hi_ocr = _group_seqs(
                rg[ishg], cg[ishg], T_ahi[g], T_ovhi[g], SPLIT
            )
            g_lo_rows.append(lo_rows)
            g_hi_rows.append(hi_rows)
            g_lo_cr.append(np.concatenate([lo_acr, lo_ocr]))
            g_hi_cr.append(np.concatenate([hi_acr, hi_ocr]))
            g_ov_cr.append(np.concatenate([lo_ocr, hi_ocr]))

        # chunk-ordered streams: per chunk [lo tiles of g..][hi tiles of g..]
        seq_lo = [g_lo_rows[g] for g in range(NG)]
        seq_hi = [g_hi_rows[g] for g in range(NG)]
        seq_cr = []        # tile order = s order (per chunk: lo.., hi..)
        for (ga, gb) in chunks:
            for g in range(ga, gb):
                seq_cr.append(g_lo_cr[g])
            for g in range(ga, gb):
                seq_cr.append(g_hi_cr[g])
        scr = np.concatenate(seq_cr)
        assert len(scr) == NT * P, (len(scr), NT * P)
        scr_t = scr.reshape(NT, P)
        # patterns for ALL tiles in s order: identity for aligned slots
        # (colrel == slot index), one-hot rows otherwise; padding -> zeros
        ohs = (scr_t[:, :, None] == iota_row[None, :, :]).astype(BF16)
        # ohT for overflow tiles only, in (chunk, group, ovlo, ovhi) order
        seq_ovcr = []
        for (ga, gb) in chunks:
            for g in range(ga, gb):
                seq_ovcr.append(g_ov_cr[g])
        ovcr = np.concatenate(seq_ovcr) if NTb else np.zeros(0, np.float32)
        ovcr_t = ovcr.reshape(max(NTb, 1), P) if NTb else np.zeros(
            (1, P), np.float32
        )
        # ohT[t, dst, slot] = 1 iff ovcr[t, slot] == dst
        iota_dst = np.arange(P, dtype=np.float32)[None, :, None]
        ohT = (ovcr_t[:, None, :] == iota_dst).astype(BF16) if NTb else (
            np.zeros((1, P, P), BF16)
        )
        slo = np.concatenate(seq_lo)
        shi = np.concatenate(seq_hi)
        im = dict(shared)
        im["xT"] = np.ascontiguousarray(xp[k * NLOC : (k + 1) * NLOC].T)
        im["ixlo"] = _wrap16(slo)
        im["ixhi"] = _wrap16(shi)
        im["ohp"] = np.ascontiguousarray(
            ohs.transpose(1, 0, 2).reshape(P, NT * P)
        )
        im["ohTp"] = np.ascontiguousarray(
            ohT.transpose(1, 0, 2).reshape(P, max(NTb, 1) * P)
        )
        im["maskb"] = np.ascontiguousarray(
            np.where(scr_t.T < 0, -30.0, 0.0).astype(np.float32)
        )
        in_maps.append(im)

    if _trace:
        _install_ntff_hook()
    res = run_bass_kernel_spmd(
        nc, in_maps, core_ids=list(range(N_CORES)), trace=_trace
    )
    out = np.concatenate(
        [res.results[k]["out"] for k in range(N_CORES)], axis=0
    )[:N]
    if _trace:
        kernel.last_exec_time_ns = res.exec_time_ns
    return np.ascontiguousarray(out, dtype=np.float32)
